# revision 23
# baseline (speedup 1.0000x reference)
"""AtomSelector Trainium2 kernel.

For each (n, l) pair pick the position of the first atom j (along the A=37
atom axis) with mask_atoms[n,l,j] != 0 and target_mask[l,j] != 0, else 0;
plus a validity mask.

Sharding: pure data parallel over N (128 samples) -> 16 samples per core on 8
cores; target_mask replicated. No cross-device communication.

Per-core algorithm (rows = n*L + l; partition p owns T=32 consecutive rows per
tile, tile = 2 samples = 4096 rows, 8 tiles per core):
  key[t,a] = mask[t,a] * target[t,a] * (37-a)     (int32; unique nonzero values)
  mx[t]    = max_a key[t,a]                        (segmented reduce, axis X)
  w[t,a]   = (key[t,a] == mx[t])                   (one-hot at first hit if mx>0)
  wp       = pos * broadcast(w)                    (single fused TT, stride-0 bcast)
  po[t,c]  = sum_a wp[t,a,c]                       (segmented reduce via (t,c,a) AP)
  mo[t]    = mx[t] >= 1
  po[t,:] *= mo[t]                                 (kills the mx==0 all-ones w case)

Engines: gpsimd does key/w/mo/fix; DVE does the two big reduces and the mul.
"""

import numpy as np
from contextlib import ExitStack

import concourse.bass as bass
import concourse.bacc as bacc
import concourse.tile as tile
import concourse.mybir as mybir
from concourse.bass_utils import run_bass_kernel_spmd

N, L, A = 128, 2048, 37
NCORES = 8
NSAMP = N // NCORES       # samples per core
T = 32                    # rows per partition per tile
SPT = (128 * T) // L      # samples per tile = 2
NT = NSAMP // SPT         # tiles per core = 8
ROWS = NSAMP * L          # rows per core

_CACHED = {}


def _build_nc(repeat=1):
    """repeat>1 wraps the whole pipeline in a tc.For_i loop (for timing via
    slope between two repeat counts); the body is emitted once."""
    nc = bacc.Bacc("TRN2", target_bir_lowering=False, debug=False)

    pos_d = nc.dram_tensor("pos", [ROWS, A * 3], mybir.dt.float32, kind="ExternalInput")
    mask_d = nc.dram_tensor("mask", [ROWS, A], mybir.dt.int32, kind="ExternalInput")
    targ_d = nc.dram_tensor("targ", [L, A], mybir.dt.int32, kind="ExternalInput")
    po_d = nc.dram_tensor("po", [ROWS, 3], mybir.dt.float32, kind="ExternalOutput")
    mo_d = nc.dram_tensor("mo", [ROWS, 1], mybir.dt.float32, kind="ExternalOutput")

    FA = T * A            # mask free elems per partition per tile
    FP = T * A * 3        # pos free elems

    with tile.TileContext(nc) as tc:
        with ExitStack() as ctx:
            cpool = ctx.enter_context(tc.tile_pool(name="const", bufs=1))
            opool = ctx.enter_context(tc.tile_pool(name="outs", bufs=1))
            wpool = ctx.enter_context(tc.tile_pool(name="work", bufs=3))
            spool = ctx.enter_context(tc.tile_pool(name="small", bufs=5))

            # ---- constants ----
            # tgk[p, t, a] = target[(T*p + t) % L, a] * (37-a); the (37-a)
            # scaling is host-side, so this is just two replicating DMAs.
            tgk = cpool.tile([128, FA], mybir.dt.int32)
            half = L // T                      # partitions before l wraps (64)
            for h in range(128 // half):
                nc.sync.dma_start(
                    tgk[h * half:(h + 1) * half, :],
                    bass.AP(targ_d, 0, [[T * A, half], [1, T * A]]),
                )

            # ---- resident output staging ----
            PO = opool.tile([128, NT * T * 3], mybir.dt.float32)
            MO = opool.tile([128, NT * T], mybir.dt.float32)
            PO4 = PO[:].rearrange("p (k t c) -> p k t c", t=T, c=3)
            MO3 = MO[:].rearrange("p (k t) -> p k t", t=T)

            rep_ctx = tc.For_i(0, repeat, 1) if repeat > 1 else None
            if rep_ctx is not None:
                ctx.enter_context(rep_ctx)
            # Short chunks at the head (and tail) shorten pipeline fill/drain;
            # full-size chunks in the middle keep per-instruction overhead low.
            chunks = [(0, 0, 8), (0, 8, 8), (0, 16, 8), (0, 24, 8)]
            chunks += [(k, 0, T) for k in range(1, NT - 1)]
            chunks += [(NT - 1, 0, 16), (NT - 1, 16, 8), (NT - 1, 24, 8)]
            if True:
                for (k, t0, tl) in chunks:
                    fa, fp = tl * A, tl * A * 3
                    M = spool.tile([128, fa], mybir.dt.int32, tag="m")
                    nc.sync.dma_start(
                        M[:],
                        bass.AP(mask_d, (k * 128 * T + t0) * A,
                                [[T * A, 128], [1, fa]]),
                    )
                    P = wpool.tile([128, fp], mybir.dt.float32, tag="p")
                    nc.sync.dma_start(
                        P[:],
                        bass.AP(pos_d, (k * 128 * T + t0) * A * 3,
                                [[T * A * 3, 128], [1, fp]]),
                    )

                    K = spool.tile([128, fa], mybir.dt.int32, tag="k")
                    K3 = K[:].rearrange("p (t a) -> p t a", a=A)
                    tgks = tgk[:].rearrange("p (t a) -> p t a", a=A)[:, t0:t0 + tl, :]
                    M3 = M[:].rearrange("p (t a) -> p t a", a=A)
                    nc.gpsimd.tensor_tensor(K3, M3, tgks, op=mybir.AluOpType.mult)

                    mx = spool.tile([128, tl], mybir.dt.int32, tag="mx")
                    nc.vector.tensor_reduce(mx[:].unsqueeze(2), K3,
                                            axis=mybir.AxisListType.X,
                                            op=mybir.AluOpType.max)

                    # D = key - mx  (<= 0, == 0 exactly at the first hit)
                    D = spool.tile([128, fa], mybir.dt.int32, tag="d")
                    D3 = D[:].rearrange("p (t a) -> p t a", a=A)
                    mxb = mx[:].unsqueeze(2).broadcast_to([128, tl, A])
                    nc.gpsimd.tensor_tensor(D3, K3, mxb,
                                            op=mybir.AluOpType.subtract)

                    # W = relu(D + 1) in {0.0, 1.0} (one-hot; all-ones iff mx==0)
                    W = spool.tile([128, fa], mybir.dt.float32, tag="w")
                    W3 = W[:].rearrange("p (t a) -> p t a", a=A)
                    nc.scalar.activation(W[:], D[:],
                                         mybir.ActivationFunctionType.Relu,
                                         bias=1.0, scale=1.0)

                    WP = wpool.tile([128, fp], mybir.dt.float32, tag="wp")
                    WP4 = WP[:].rearrange("p (t a c) -> p t a c", a=A, c=3)
                    P4 = P[:].rearrange("p (t a c) -> p t a c", a=A, c=3)
                    Wb = W3.unsqueeze(3).broadcast_to([128, tl, A, 3])
                    nc.vector.tensor_tensor(WP4, P4, Wb, op=mybir.AluOpType.mult)

                    WPr = WP[:].rearrange("p (t a c) -> p t c a", a=A, c=3)
                    posl = PO4[:, k, t0:t0 + tl, :]
                    nc.vector.tensor_reduce(posl.unsqueeze(3), WPr,
                                            axis=mybir.AxisListType.X,
                                            op=mybir.AluOpType.add)

                    mosl = MO3[:, k, t0:t0 + tl]
                    nc.scalar.sign(mosl, mx[:])

                    mob = mosl.unsqueeze(2).broadcast_to([128, tl, 3])
                    nc.gpsimd.tensor_tensor(posl, posl, mob,
                                            op=mybir.AluOpType.mult)

                    # drain the first half of the outputs mid-stream so the
                    # final DMA isn't all serialized into the tail
                    if k == NT // 2 - 1 and t0 + tl == T:
                        nc.sync.dma_start(
                            bass.AP(po_d, 0,
                                    [[T * 3, 128], [128 * T * 3, NT // 2],
                                     [3, T], [1, 3]]),
                            PO4[:, 0:NT // 2, :, :],
                        )
                        nc.sync.dma_start(
                            bass.AP(mo_d, 0,
                                    [[T, 128], [128 * T, NT // 2], [1, T]]),
                            MO3[:, 0:NT // 2, :],
                        )

                # ---- final output DMAs (second half) ----
                nc.sync.dma_start(
                    bass.AP(po_d, 128 * T * 3 * (NT // 2),
                            [[T * 3, 128], [128 * T * 3, NT - NT // 2],
                             [3, T], [1, 3]]),
                    PO4[:, NT // 2:, :, :],
                )
                nc.sync.dma_start(
                    bass.AP(mo_d, 128 * T * (NT // 2),
                            [[T, 128], [128 * T, NT - NT // 2], [1, T]]),
                    MO3[:, NT // 2:, :],
                )

    nc.compile()
    return nc


def _get_nc(repeat=1):
    key = ("nc", repeat)
    if key not in _CACHED:
        _CACHED[key] = _build_nc(repeat)
    return _CACHED[key]


def run(pos_atoms, mask_atoms, target_mask, trace=False, repeat=1):
    pos = np.ascontiguousarray(pos_atoms, dtype=np.float32).reshape(N * L, A * 3)
    mask = np.ascontiguousarray(mask_atoms, dtype=np.int32).reshape(N * L, A)
    targ = np.ascontiguousarray(target_mask, dtype=np.int32).reshape(L, A)
    targ = (targ * (A - np.arange(A, dtype=np.int32))[None, :]).astype(np.int32)

    nc = _get_nc(repeat)
    in_maps = [
        {
            "pos": pos[c * ROWS:(c + 1) * ROWS],
            "mask": mask[c * ROWS:(c + 1) * ROWS],
            "targ": targ,
        }
        for c in range(NCORES)
    ]
    res = run_bass_kernel_spmd(nc, in_maps, core_ids=list(range(NCORES)),
                               trace=trace)
    po = np.concatenate([r["po"] for r in res.results], axis=0)
    mo = np.concatenate([r["mo"] for r in res.results], axis=0)
    pos_out = po.reshape(N, L, 3)
    mask_out = mo.reshape(N, L)
    return (pos_out, mask_out), res


def kernel(pos_atoms, mask_atoms, target_mask):
    (pos_out, mask_out), _ = run(pos_atoms, mask_atoms, target_mask, trace=False)
    return pos_out, mask_out


# ---------------------------------------------------------------------------
# Timing support (test.py only): cached PJRT executable + repeat-count slope.
# ---------------------------------------------------------------------------

def _make_runner(nc):
    """Build the shard_map'd jitted callable for `nc` once (mirrors
    bass2jax.run_bass_via_pjrt) and return (fn, in_names, out_names, zeros)."""
    import jax
    import numpy as _np
    from jax.sharding import Mesh, PartitionSpec
    from jax.experimental.shard_map import shard_map
    from concourse import bass2jax
    from concourse import mybir as mb

    bass2jax.install_neuronx_cc_hook()
    partition_name = nc.partition_id_tensor.name if nc.partition_id_tensor else None
    in_names, out_names, out_avals, zero_outs = [], [], [], []
    for alloc in nc.m.functions[0].allocations:
        if not isinstance(alloc, mb.MemoryLocationSet):
            continue
        name = alloc.memorylocations[0].name
        if alloc.kind == "ExternalInput":
            if name != partition_name:
                in_names.append(name)
        elif alloc.kind == "ExternalOutput":
            out_names.append(name)
            shape = tuple(alloc.tensor_shape)
            dtype = mb.dt.np(alloc.dtype)
            out_avals.append(jax.core.ShapedArray(shape, dtype))
            zero_outs.append(_np.zeros(shape, dtype))
    n_params = len(in_names)
    all_names = in_names + out_names
    if partition_name is not None:
        all_names = all_names + [partition_name]

    def _body(*args):
        operands = list(args)
        if partition_name is not None:
            operands.append(bass2jax.partition_id_tensor())
        outs = bass2jax._bass_exec_p.bind(
            *operands,
            out_avals=tuple(out_avals),
            in_names=tuple(all_names),
            out_names=tuple(out_names),
            lowering_input_output_aliases=(),
            sim_require_finite=True,
            sim_require_nnan=True,
            nc=nc,
        )
        return tuple(outs)

    devices = jax.devices()[:NCORES]
    mesh = Mesh(np.asarray(devices), ("core",))
    nin = n_params + len(out_names)
    fn = jax.jit(
        shard_map(_body, mesh=mesh,
                  in_specs=(PartitionSpec("core"),) * nin,
                  out_specs=(PartitionSpec("core"),) * len(out_names),
                  check_rep=False),
        keep_unused=True,
    )
    return fn, in_names, out_names, zero_outs


def _concat_inputs(in_maps, in_names):
    return [np.concatenate([np.asarray(in_maps[c][n]) for c in range(NCORES)],
                           axis=0) for n in in_names]


def measure_exec_ns(pos_atoms, mask_atoms, target_mask, r1=32, r2=1056,
                    iters=7):
    """Per-"kernel job" device time in ns via repeat-count slope."""
    import time
    import jax

    pos = np.ascontiguousarray(pos_atoms, dtype=np.float32).reshape(N * L, A * 3)
    mask = np.ascontiguousarray(mask_atoms, dtype=np.int32).reshape(N * L, A)
    targ = np.ascontiguousarray(target_mask, dtype=np.int32).reshape(L, A)
    targ = (targ * (A - np.arange(A, dtype=np.int32))[None, :]).astype(np.int32)
    in_maps = [
        {"pos": pos[c * ROWS:(c + 1) * ROWS],
         "mask": mask[c * ROWS:(c + 1) * ROWS],
         "targ": targ}
        for c in range(NCORES)
    ]

    results = {}
    for r in (r1, r2):
        nc = _get_nc(r)
        fn, in_names, out_names, zero_outs = _make_runner(nc)
        concat_in = _concat_inputs(in_maps, in_names)
        concat_zeros = [np.zeros((NCORES * z.shape[0], *z.shape[1:]), z.dtype)
                        for z in zero_outs]
        args = [jax.device_put(a) for a in concat_in] + \
               [jax.device_put(z) for z in concat_zeros]
        jax.block_until_ready(fn(*args))  # warm-up / compile
        times = []
        for _ in range(iters):
            t0 = time.perf_counter()
            jax.block_until_ready(fn(*args))
            times.append(time.perf_counter() - t0)
        results[r] = min(times)
        print(f"  repeat={r}: min call {results[r]*1e3:.1f} ms "
              f"(all: {[f'{t*1e3:.0f}' for t in times]})")
    slope_ns = (results[r2] - results[r1]) / (r2 - r1) * 1e9
    return slope_ns


# revision 24
# speedup vs baseline: 1.9783x; 1.9783x over previous
"""AtomSelector Trainium2 kernel.

For each (n, l) pair pick the position of the first atom j (along the A=37
atom axis) with mask_atoms[n,l,j] != 0 and target_mask[l,j] != 0, else 0;
plus a validity mask.

Sharding: pure data parallel over N (128 samples) -> 16 samples per core on 8
cores; target_mask replicated. No cross-device communication.

Per-core algorithm (rows = n*L + l; partition p owns T=32 consecutive rows per
tile, tile = 2 samples = 4096 rows, 8 tiles per core):
  key[t,a] = mask[t,a] * target[t,a] * (37-a)     (int32; unique nonzero values)
  mx[t]    = max_a key[t,a]                        (segmented reduce, axis X)
  w[t,a]   = (key[t,a] == mx[t])                   (one-hot at first hit if mx>0)
  wp       = pos * broadcast(w)                    (single fused TT, stride-0 bcast)
  po[t,c]  = sum_a wp[t,a,c]                       (segmented reduce via (t,c,a) AP)
  mo[t]    = mx[t] >= 1
  po[t,:] *= mo[t]                                 (kills the mx==0 all-ones w case)

Engines: gpsimd does key/w/mo/fix; DVE does the two big reduces and the mul.
"""

import numpy as np
from contextlib import ExitStack

import concourse.bass as bass
import concourse.bacc as bacc
import concourse.tile as tile
import concourse.mybir as mybir
from concourse.bass_utils import run_bass_kernel_spmd

N, L, A = 128, 2048, 37
NCORES = 8
NSAMP = N // NCORES       # samples per core
T = 32                    # rows per partition per tile
SPT = (128 * T) // L      # samples per tile = 2
NT = NSAMP // SPT         # tiles per core = 8
ROWS = NSAMP * L          # rows per core

_CACHED = {}


def _build_nc(repeat=1):
    """repeat>1 wraps the whole pipeline in a tc.For_i loop (for timing via
    slope between two repeat counts); the body is emitted once."""
    nc = bacc.Bacc("TRN2", target_bir_lowering=False, debug=False)

    pos_d = nc.dram_tensor("pos", [ROWS, A * 3], mybir.dt.float32, kind="ExternalInput")
    mask_d = nc.dram_tensor("mask", [ROWS, A], mybir.dt.int32, kind="ExternalInput")
    targ_d = nc.dram_tensor("targ", [L, A], mybir.dt.int32, kind="ExternalInput")
    po_d = nc.dram_tensor("po", [ROWS, 3], mybir.dt.float32, kind="ExternalOutput")
    mo_d = nc.dram_tensor("mo", [ROWS, 1], mybir.dt.float32, kind="ExternalOutput")

    FA = T * A            # mask free elems per partition per tile
    FP = T * A * 3        # pos free elems

    with tile.TileContext(nc) as tc:
        with ExitStack() as ctx:
            cpool = ctx.enter_context(tc.tile_pool(name="const", bufs=1))
            opool = ctx.enter_context(tc.tile_pool(name="outs", bufs=1))
            wpool = ctx.enter_context(tc.tile_pool(name="work", bufs=3))
            spool = ctx.enter_context(tc.tile_pool(name="small", bufs=5))

            # ---- constants ----
            # tgk[p, t, a] = target[(T*p + t) % L, a] * (37-a); the (37-a)
            # scaling is host-side, so this is just two replicating DMAs.
            tgk = cpool.tile([128, FA], mybir.dt.int32)
            half = L // T                      # partitions before l wraps (64)
            for h in range(128 // half):
                nc.sync.dma_start(
                    tgk[h * half:(h + 1) * half, :],
                    bass.AP(targ_d, 0, [[T * A, half], [1, T * A]]),
                )

            # ---- resident output staging ----
            PO = opool.tile([128, NT * T * 3], mybir.dt.float32)
            MO = opool.tile([128, NT * T], mybir.dt.float32)
            PO4 = PO[:].rearrange("p (k t c) -> p k t c", t=T, c=3)
            MO3 = MO[:].rearrange("p (k t) -> p k t", t=T)

            rep_ctx = tc.For_i(0, repeat, 1) if repeat > 1 else None
            if rep_ctx is not None:
                ctx.enter_context(rep_ctx)
            # Short chunks at the head (and tail) shorten pipeline fill/drain;
            # full-size chunks in the middle keep per-instruction overhead low.
            chunks = [(0, 0, 8), (0, 8, 8), (0, 16, 8), (0, 24, 8)]
            chunks += [(k, 0, T) for k in range(1, NT - 1)]
            chunks += [(NT - 1, 0, 16), (NT - 1, 16, 8), (NT - 1, 24, 8)]
            if True:
                for (k, t0, tl) in chunks:
                    fa, fp = tl * A, tl * A * 3
                    M = spool.tile([128, fa], mybir.dt.int32, tag="m")
                    nc.sync.dma_start(
                        M[:],
                        bass.AP(mask_d, (k * 128 * T + t0) * A,
                                [[T * A, 128], [1, fa]]),
                    )
                    P = wpool.tile([128, fp], mybir.dt.float32, tag="p")
                    nc.sync.dma_start(
                        P[:],
                        bass.AP(pos_d, (k * 128 * T + t0) * A * 3,
                                [[T * A * 3, 128], [1, fp]]),
                    )

                    K = spool.tile([128, fa], mybir.dt.int32, tag="k")
                    K3 = K[:].rearrange("p (t a) -> p t a", a=A)
                    tgks = tgk[:].rearrange("p (t a) -> p t a", a=A)[:, t0:t0 + tl, :]
                    M3 = M[:].rearrange("p (t a) -> p t a", a=A)
                    nc.gpsimd.tensor_tensor(K3, M3, tgks, op=mybir.AluOpType.mult)

                    mx = spool.tile([128, tl], mybir.dt.int32, tag="mx")
                    nc.vector.tensor_reduce(mx[:].unsqueeze(2), K3,
                                            axis=mybir.AxisListType.X,
                                            op=mybir.AluOpType.max)

                    # D = key - mx  (<= 0, == 0 exactly at the first hit)
                    D = spool.tile([128, fa], mybir.dt.int32, tag="d")
                    D3 = D[:].rearrange("p (t a) -> p t a", a=A)
                    mxb = mx[:].unsqueeze(2).broadcast_to([128, tl, A])
                    nc.gpsimd.tensor_tensor(D3, K3, mxb,
                                            op=mybir.AluOpType.subtract)

                    # W = relu(D + 1) in {0.0, 1.0} (one-hot; all-ones iff mx==0)
                    W = spool.tile([128, fa], mybir.dt.float32, tag="w")
                    W3 = W[:].rearrange("p (t a) -> p t a", a=A)
                    nc.scalar.activation(W[:], D[:],
                                         mybir.ActivationFunctionType.Relu,
                                         bias=1.0, scale=1.0)

                    WP = wpool.tile([128, fp], mybir.dt.float32, tag="wp")
                    WP4 = WP[:].rearrange("p (t a c) -> p t a c", a=A, c=3)
                    P4 = P[:].rearrange("p (t a c) -> p t a c", a=A, c=3)
                    Wb = W3.unsqueeze(3).broadcast_to([128, tl, A, 3])
                    nc.vector.tensor_tensor(WP4, P4, Wb, op=mybir.AluOpType.mult)

                    WPr = WP[:].rearrange("p (t a c) -> p t c a", a=A, c=3)
                    posl = PO4[:, k, t0:t0 + tl, :]
                    nc.vector.tensor_reduce(posl.unsqueeze(3), WPr,
                                            axis=mybir.AxisListType.X,
                                            op=mybir.AluOpType.add)

                    mosl = MO3[:, k, t0:t0 + tl]
                    nc.scalar.sign(mosl, mx[:])

                    mob = mosl.unsqueeze(2).broadcast_to([128, tl, 3])
                    nc.gpsimd.tensor_tensor(posl, posl, mob,
                                            op=mybir.AluOpType.mult)

                # ---- final output DMAs ----
                nc.sync.dma_start(
                    bass.AP(po_d, 0, [[T * 3, 128], [128 * T * 3, NT], [3, T], [1, 3]]),
                    PO4,
                )
                nc.sync.dma_start(
                    bass.AP(mo_d, 0, [[T, 128], [128 * T, NT], [1, T]]),
                    MO3,
                )

    nc.compile()
    return nc


def _get_nc(repeat=1):
    key = ("nc", repeat)
    if key not in _CACHED:
        _CACHED[key] = _build_nc(repeat)
    return _CACHED[key]


def run(pos_atoms, mask_atoms, target_mask, trace=False, repeat=1):
    pos = np.ascontiguousarray(pos_atoms, dtype=np.float32).reshape(N * L, A * 3)
    mask = np.ascontiguousarray(mask_atoms, dtype=np.int32).reshape(N * L, A)
    targ = np.ascontiguousarray(target_mask, dtype=np.int32).reshape(L, A)
    targ = (targ * (A - np.arange(A, dtype=np.int32))[None, :]).astype(np.int32)

    nc = _get_nc(repeat)
    in_maps = [
        {
            "pos": pos[c * ROWS:(c + 1) * ROWS],
            "mask": mask[c * ROWS:(c + 1) * ROWS],
            "targ": targ,
        }
        for c in range(NCORES)
    ]
    res = run_bass_kernel_spmd(nc, in_maps, core_ids=list(range(NCORES)),
                               trace=trace)
    po = np.concatenate([r["po"] for r in res.results], axis=0)
    mo = np.concatenate([r["mo"] for r in res.results], axis=0)
    pos_out = po.reshape(N, L, 3)
    mask_out = mo.reshape(N, L)
    return (pos_out, mask_out), res


def kernel(pos_atoms, mask_atoms, target_mask):
    (pos_out, mask_out), _ = run(pos_atoms, mask_atoms, target_mask, trace=False)
    return pos_out, mask_out


# ---------------------------------------------------------------------------
# Timing support (test.py only): cached PJRT executable + repeat-count slope.
# ---------------------------------------------------------------------------

def _make_runner(nc):
    """Build the shard_map'd jitted callable for `nc` once (mirrors
    bass2jax.run_bass_via_pjrt) and return (fn, in_names, out_names, zeros)."""
    import jax
    import numpy as _np
    from jax.sharding import Mesh, PartitionSpec
    from jax.experimental.shard_map import shard_map
    from concourse import bass2jax
    from concourse import mybir as mb

    bass2jax.install_neuronx_cc_hook()
    partition_name = nc.partition_id_tensor.name if nc.partition_id_tensor else None
    in_names, out_names, out_avals, zero_outs = [], [], [], []
    for alloc in nc.m.functions[0].allocations:
        if not isinstance(alloc, mb.MemoryLocationSet):
            continue
        name = alloc.memorylocations[0].name
        if alloc.kind == "ExternalInput":
            if name != partition_name:
                in_names.append(name)
        elif alloc.kind == "ExternalOutput":
            out_names.append(name)
            shape = tuple(alloc.tensor_shape)
            dtype = mb.dt.np(alloc.dtype)
            out_avals.append(jax.core.ShapedArray(shape, dtype))
            zero_outs.append(_np.zeros(shape, dtype))
    n_params = len(in_names)
    all_names = in_names + out_names
    if partition_name is not None:
        all_names = all_names + [partition_name]

    def _body(*args):
        operands = list(args)
        if partition_name is not None:
            operands.append(bass2jax.partition_id_tensor())
        outs = bass2jax._bass_exec_p.bind(
            *operands,
            out_avals=tuple(out_avals),
            in_names=tuple(all_names),
            out_names=tuple(out_names),
            lowering_input_output_aliases=(),
            sim_require_finite=True,
            sim_require_nnan=True,
            nc=nc,
        )
        return tuple(outs)

    devices = jax.devices()[:NCORES]
    mesh = Mesh(np.asarray(devices), ("core",))
    nin = n_params + len(out_names)
    fn = jax.jit(
        shard_map(_body, mesh=mesh,
                  in_specs=(PartitionSpec("core"),) * nin,
                  out_specs=(PartitionSpec("core"),) * len(out_names),
                  check_rep=False),
        keep_unused=True,
    )
    return fn, in_names, out_names, zero_outs


def _concat_inputs(in_maps, in_names):
    return [np.concatenate([np.asarray(in_maps[c][n]) for c in range(NCORES)],
                           axis=0) for n in in_names]


def measure_exec_ns(pos_atoms, mask_atoms, target_mask, r1=32, r2=1056,
                    iters=7):
    """Per-"kernel job" device time in ns via repeat-count slope."""
    import time
    import jax

    pos = np.ascontiguousarray(pos_atoms, dtype=np.float32).reshape(N * L, A * 3)
    mask = np.ascontiguousarray(mask_atoms, dtype=np.int32).reshape(N * L, A)
    targ = np.ascontiguousarray(target_mask, dtype=np.int32).reshape(L, A)
    targ = (targ * (A - np.arange(A, dtype=np.int32))[None, :]).astype(np.int32)
    in_maps = [
        {"pos": pos[c * ROWS:(c + 1) * ROWS],
         "mask": mask[c * ROWS:(c + 1) * ROWS],
         "targ": targ}
        for c in range(NCORES)
    ]

    results = {}
    for r in (r1, r2):
        nc = _get_nc(r)
        fn, in_names, out_names, zero_outs = _make_runner(nc)
        concat_in = _concat_inputs(in_maps, in_names)
        concat_zeros = [np.zeros((NCORES * z.shape[0], *z.shape[1:]), z.dtype)
                        for z in zero_outs]
        args = [jax.device_put(a) for a in concat_in] + \
               [jax.device_put(z) for z in concat_zeros]
        jax.block_until_ready(fn(*args))  # warm-up / compile
        times = []
        for _ in range(iters):
            t0 = time.perf_counter()
            jax.block_until_ready(fn(*args))
            times.append(time.perf_counter() - t0)
        results[r] = min(times)
        print(f"  repeat={r}: min call {results[r]*1e3:.1f} ms "
              f"(all: {[f'{t*1e3:.0f}' for t in times]})")
    slope_ns = (results[r2] - results[r1]) / (r2 - r1) * 1e9
    return slope_ns
